# revision 6
# baseline (speedup 1.0000x reference)
"""Cross-attention-concat kernel for Trainium2 (8 NeuronCores, Bass/Tile).

Math (per batch b):
  x   = concat(rgb, chm) on channels           [512, 4096]   (pixels hw = h*64+w)
  Q   = Wq x + bq                              [64, 4096]
  K   = Wk x + bk                              [64, 4096]
  V   = Wv x + bv                              [256, 4096]
  S   = Q^T K                                  [4096 hw, 4096 xy]
  A   = softmax over y within each x-group of 64 keys
  att = A V^T                                  [4096 hw, 256]
  out = Wcr att^T + bcr                        [256, 4096]

Sharding: core = (batch, H-half). The host rolls each batch's pixel axis by
2048*(core%2) so every core runs the same program with its queries at
columns 0:2048 of the rolled image; keys/values span the full image (the
attention sum is invariant to the roll since K/V/attn are permuted together
and the roll is a multiple of the y-group size 64).
"""

import numpy as np

B, C, H, W = 4, 256, 64, 64
HW = H * W               # 4096
CIN = 2 * C              # 512
CQK = C // 4             # 64
QCOLS = HW // 2          # 2048 queries per core
NSUP = QCOLS // 256      # 8 super-blocks of 256 queries (2 sub-blocks of 128)

_CACHE = {}


def _build_nc():
    import concourse.bacc as bacc
    import concourse.tile as tile
    from concourse import mybir
    from concourse.masks import make_identity

    F32 = mybir.dt.float32
    AX = mybir.AxisListType
    AF = mybir.ActivationFunctionType

    nc = bacc.Bacc("TRN2", target_bir_lowering=False, debug=False, num_devices=8)

    x_d = nc.dram_tensor("x", [CIN, HW], F32, kind="ExternalInput").ap()
    wqk_d = nc.dram_tensor("wqk", [128, 4, 128], F32, kind="ExternalInput").ap()
    wvt_d = nc.dram_tensor("wvt", [128, 4, 256], F32, kind="ExternalInput").ap()
    wcr_d = nc.dram_tensor("wcr", [128, 2, 2, 128], F32, kind="ExternalInput").ap()
    bq_d = nc.dram_tensor("bq2", [64, 1], F32, kind="ExternalInput").ap()
    bk_d = nc.dram_tensor("bk2", [64, 1], F32, kind="ExternalInput").ap()
    bv64_d = nc.dram_tensor("bv64", [128, 2], F32, kind="ExternalInput").ap()
    bcr_d = nc.dram_tensor("bcr2", [128, 2], F32, kind="ExternalInput").ap()
    out_d = nc.dram_tensor("out", [C, QCOLS], F32, kind="ExternalOutput").ap()

    with tile.TileContext(nc) as tc:
        with (
            tc.tile_pool(name="const", bufs=1) as constp,
            tc.tile_pool(name="xp", bufs=1) as xp,
            tc.tile_pool(name="qkv", bufs=1) as qkvp,
            tc.tile_pool(name="pbuf", bufs=3) as pbufp,
            tc.tile_pool(name="ptbuf", bufs=2) as ptp,
            tc.tile_pool(name="attbuf", bufs=2) as attbp,
            tc.tile_pool(name="dbuf", bufs=2) as dbufp,
            tc.tile_pool(name="obuf", bufs=2) as obufp,
        ):
            # ---- constants ----
            wqk_sb = constp.tile([128, 4, 128], F32)
            wvt_sb = constp.tile([128, 4, 256], F32)
            wcr_sb = constp.tile([128, 2, 2, 128], F32)
            bq_sb = constp.tile([64, 1], F32)
            bk_sb = constp.tile([64, 1], F32)
            bv64_sb = constp.tile([128, 2], F32)
            bcr_sb = constp.tile([128, 2], F32)
            ident = constp.tile([128, 128], F32)
            nc.sync.dma_start(out=wqk_sb, in_=wqk_d)
            nc.sync.dma_start(out=wvt_sb, in_=wvt_d)
            nc.sync.dma_start(out=wcr_sb, in_=wcr_d)
            nc.sync.dma_start(out=bq_sb, in_=bq_d)
            nc.sync.dma_start(out=bk_sb, in_=bk_d)
            nc.sync.dma_start(out=bv64_sb, in_=bv64_d)
            nc.sync.dma_start(out=bcr_sb, in_=bcr_d)
            make_identity(nc, ident)

            # ---- load x as 4 partition-tiles of [128, 4096] ----
            x_sb = []
            for k in range(4):
                xk = xp.tile([128, HW], F32, tag=f"x{k}", name=f"x{k}")
                for j in range(2):
                    nc.sync.dma_start(
                        out=xk[:, j * 2048 : (j + 1) * 2048],
                        in_=x_d[k * 128 : (k + 1) * 128, j * 2048 : (j + 1) * 2048],
                    )
                x_sb.append(xk)

            q_sb = qkvp.tile([64, QCOLS], F32)    # Q for this core's queries
            k_sb = qkvp.tile([64, HW], F32)       # K, full image
            vt_sb = qkvp.tile([128, 32, 256], F32)  # V^T, [xy-block, 128, 256]

            # ---- preamble: Q, K, V^T projections ----
            with tc.tile_pool(name="ps_pre", bufs=4, space="PSUM") as ps_pre, \
                 tc.tile_pool(name="ps_prek", bufs=2, space="PSUM") as ps_prek:
                # Q over this core's 2048 query columns
                for n in range(4):
                    psq = ps_pre.tile([64, 512], F32, tag="pre", name="psq")
                    for k in range(4):
                        nc.tensor.matmul(
                            psq,
                            lhsT=wqk_sb[:, k, 0:64],
                            rhs=x_sb[k][:, n * 512 : (n + 1) * 512],
                            start=(k == 0),
                            stop=(k == 3),
                        )
                    nc.scalar.add(q_sb[:, n * 512 : (n + 1) * 512], psq, bq_sb)
                # K over the full image, in 1024-col pairs
                for n in range(4):
                    psk = ps_prek.tile([64, 1024], F32, tag="prek", name="psk")
                    for j in range(2):
                        for k in range(4):
                            nc.tensor.matmul(
                                psk[:, j * 512 : (j + 1) * 512],
                                lhsT=wqk_sb[:, k, 64:128],
                                rhs=x_sb[k][:, (2 * n + j) * 512 : (2 * n + j + 1) * 512],
                                start=(k == 0),
                                stop=(k == 3),
                            )
                    nc.vector.tensor_scalar_add(
                        k_sb[:, n * 1024 : (n + 1) * 1024], psk, bk_sb
                    )
                # V^T in xy-block pairs: out [xy 128, c 256]
                for i2 in range(16):
                    psv = ps_pre.tile([128, 512], F32, tag="pre", name="psv")
                    for j in range(2):
                        i = 2 * i2 + j
                        for k in range(4):
                            nc.tensor.matmul(
                                psv[:, j * 256 : (j + 1) * 256],
                                lhsT=x_sb[k][:, i * 128 : (i + 1) * 128],
                                rhs=wvt_sb[:, k, :],
                                start=(k == 0),
                                stop=(k == 3),
                            )
                    dst = vt_sb[:, 2 * i2 : 2 * i2 + 2, :]
                    if i2 % 2 == 0:
                        nc.scalar.copy(dst, psv)
                    else:
                        nc.vector.tensor_copy(dst, psv)

            # ---- main loop over super-blocks of 256 queries ----
            with (
                tc.tile_pool(name="ps_sc", bufs=1, space="PSUM") as ps_sc,
                tc.tile_pool(name="ps_tp", bufs=2, space="PSUM") as ps_tp,
                tc.tile_pool(name="ps_att", bufs=1, space="PSUM") as ps_att,
                tc.tile_pool(name="ps_fin", bufs=1, space="PSUM") as ps_fin,
            ):
                for S in range(NSUP):
                    p_tiles = []
                    for b in range(2):
                        hw0 = (2 * S + b) * 128
                        p_b = pbufp.tile([128, HW], F32, tag="p", name="p_b")
                        d_b = dbufp.tile([128, 64], F32, tag="d", name="d_b")
                        dr_b = dbufp.tile([128, 64], F32, tag="dr", name="dr_b")
                        for n2 in range(4):
                            pssc = ps_sc.tile([128, 1024], F32, tag="sc", name="pssc")
                            for j in range(2):
                                nc.tensor.matmul(
                                    pssc[:, j * 512 : (j + 1) * 512],
                                    lhsT=q_sb[:, hw0 : hw0 + 128],
                                    rhs=k_sb[:, (2 * n2 + j) * 512 : (2 * n2 + j + 1) * 512],
                                    start=True,
                                    stop=True,
                                )
                            nc.scalar.activation(
                                p_b[:, n2 * 1024 : (n2 + 1) * 1024], pssc, AF.Exp
                            )
                            nc.vector.reduce_sum(
                                out=d_b[:, n2 * 16 : (n2 + 1) * 16],
                                in_=p_b[:, n2 * 1024 : (n2 + 1) * 1024].rearrange(
                                    "p (x y) -> p x y", y=64
                                ),
                                axis=AX.X,
                            )
                        nc.vector.reciprocal(dr_b, d_b)
                        # attn = exp(s) / D with D broadcast over each y-group
                        dr_bc = dr_b.unsqueeze(2).broadcast_to((128, 64, 64))
                        p3 = p_b.rearrange("p (x y) -> p x y", y=64)
                        nc.gpsimd.tensor_mul(p3, p3, dr_bc)
                        p_tiles.append(p_b)

                    # transpose attn into [xy, hw] packs of 2 chunks x 2 subs
                    pt_tiles = []
                    for m in range(16):
                        pstp = ps_tp.tile([128, 512], F32, tag="tp", name="pstp")
                        for jj in range(2):
                            i = 2 * m + jj
                            for b in range(2):
                                nc.tensor.transpose(
                                    pstp[:, jj * 256 + b * 128 : jj * 256 + b * 128 + 128],
                                    p_tiles[b][:, i * 128 : (i + 1) * 128],
                                    ident,
                                )
                        pt = ptp.tile([128, 512], F32, tag="pt", name="pt")
                        if m % 2 == 0:
                            nc.scalar.copy(pt, pstp)
                        else:
                            nc.vector.tensor_copy(pt, pstp)
                        pt_tiles.append(pt)

                    # attended^T [c, hw]: accumulate over all 32 xy-chunks.
                    # One psum tile (bank) per c-half so the two accumulation
                    # groups never interleave within a bank.
                    att_h = [
                        ps_att.tile([128, 256], F32, tag=f"att{h}", name=f"att{h}")
                        for h in range(2)
                    ]
                    for i in range(32):
                        m, jj = divmod(i, 2)
                        rhs = pt_tiles[m][:, jj * 256 : (jj + 1) * 256]
                        for h in range(2):
                            nc.tensor.matmul(
                                att_h[h],
                                lhsT=vt_sb[:, i, h * 128 : (h + 1) * 128],
                                rhs=rhs,
                                start=(i == 0),
                                stop=(i == 31),
                            )
                    # + V-bias: sum_xy attn = 64 exactly, so bias term = 64*bv
                    attT = attbp.tile([128, 512], F32, tag="attT", name="attT")
                    nc.scalar.add(attT[:, 0:256], att_h[0], bv64_sb[:, 0:1])
                    nc.vector.tensor_scalar_add(
                        attT[:, 256:512], att_h[1], bv64_sb[:, 1:2]
                    )

                    # final projection out[co, hw] = Wcr @ att^T + bcr
                    # (g-outer so each accumulation group is contiguous)
                    psf = ps_fin.tile([128, 512], F32, tag="fin", name="psf")
                    for g in range(2):
                        for h in range(2):
                            nc.tensor.matmul(
                                psf[:, g * 256 : (g + 1) * 256],
                                lhsT=wcr_sb[:, h, g, :],
                                rhs=attT[:, h * 256 : (h + 1) * 256],
                                start=(h == 0),
                                stop=(h == 1),
                            )
                    out_t = obufp.tile([128, 512], F32, tag="out_t", name="out_t")
                    nc.scalar.add(out_t[:, 0:256], psf[:, 0:256], bcr_sb[:, 0:1])
                    nc.vector.tensor_scalar_add(
                        out_t[:, 256:512], psf[:, 256:512], bcr_sb[:, 1:2]
                    )
                    for g in range(2):
                        nc.sync.dma_start(
                            out=out_d[g * 128 : (g + 1) * 128, S * 256 : (S + 1) * 256],
                            in_=out_t[:, g * 256 : (g + 1) * 256],
                        )
    nc.compile()
    return nc


def get_nc():
    if "nc" not in _CACHE:
        _CACHE["nc"] = _build_nc()
    return _CACHE["nc"]


def make_in_maps(inputs):
    rgb = np.asarray(inputs["rgb_features"], np.float32)
    chm = np.asarray(inputs["chm_features"], np.float32)
    Wq = np.asarray(inputs["Wq"], np.float32)
    bq = np.asarray(inputs["bq"], np.float32)
    Wk = np.asarray(inputs["Wk"], np.float32)
    bk = np.asarray(inputs["bk"], np.float32)
    Wv = np.asarray(inputs["Wv"], np.float32)
    bv = np.asarray(inputs["bv"], np.float32)
    Wcr = np.asarray(inputs["Wcr"], np.float32)
    bcr = np.asarray(inputs["bcr"], np.float32)

    Wqk = np.concatenate([Wq, Wk], axis=0)  # [128, 512]
    wqk = np.ascontiguousarray(Wqk.T.reshape(4, 128, 128).transpose(1, 0, 2))
    wvt = np.ascontiguousarray(Wv.T.reshape(4, 128, 256).transpose(1, 0, 2))
    wcr = np.ascontiguousarray(Wcr.T.reshape(2, 128, 2, 128).transpose(1, 0, 2, 3))
    bq2 = np.ascontiguousarray(bq.reshape(64, 1))
    bk2 = np.ascontiguousarray(bk.reshape(64, 1))
    bv64 = np.ascontiguousarray((64.0 * bv).reshape(2, 128).T)
    bcr2 = np.ascontiguousarray(bcr.reshape(2, 128).T)

    in_maps = []
    for core in range(8):
        b, par = divmod(core, 2)
        x = np.concatenate([rgb[b], chm[b]], axis=0).reshape(CIN, HW)
        if par:
            x = np.roll(x, -QCOLS, axis=1)
        in_maps.append(
            {
                "x": np.ascontiguousarray(x),
                "wqk": wqk,
                "wvt": wvt,
                "wcr": wcr,
                "bq2": bq2,
                "bk2": bk2,
                "bv64": bv64,
                "bcr2": bcr2,
            }
        )
    return in_maps


def assemble(outs):
    full = np.empty((B, C, HW), np.float32)
    for core in range(8):
        b, par = divmod(core, 2)
        full[b, :, par * QCOLS : (par + 1) * QCOLS] = outs[core]
    return full.reshape(B, C, H, W)


def kernel(**inputs):
    from concourse.bass_utils import run_bass_kernel_spmd

    nc = get_nc()
    res = run_bass_kernel_spmd(nc, make_in_maps(inputs), core_ids=list(range(8)))
    return assemble([r["out"] for r in res.results])


# revision 7
# speedup vs baseline: 1.4294x; 1.4294x over previous
"""Cross-attention-concat kernel for Trainium2 (8 NeuronCores, Bass/Tile).

Math (per batch b):
  x   = concat(rgb, chm) on channels           [512, 4096]   (pixels hw = h*64+w)
  Q   = Wq x + bq ; K = Wk x + bk              [64, ...]
  V   = Wv x + bv                              [256, 4096]
  S   = Q^T K                                  [2048 hw, 4096 xy]
  A   = softmax over y within each x-group of 64 keys
  out = Wcr (A V^T)^T + bcr                    [256, 2048]

Sharding: core = (batch, H-half). The host rolls each batch's pixel axis by
2048*(core%2) so every core runs the same program with its queries at
columns 0:2048 of the rolled image (attention is invariant to the roll:
K/V/attn permute together and the roll is a multiple of the y-group 64).

Precision: scores/softmax-denominator path is fp32 (exp amplifies input
error); the attention matrix and V are bf16 (PE streams bf16 at 1 col/cycle
vs 2 for fp32, with fp32 PSUM accumulation), final projection fp32.
"""

import numpy as np
import ml_dtypes

B, C, H, W = 4, 256, 64, 64
HW = H * W               # 4096
CIN = 2 * C              # 512
QCOLS = HW // 2          # 2048 queries per core
NSUP = QCOLS // 512      # 4 super-blocks of 512 queries (4 sub-blocks of 128)

_CACHE = {}


def _build_nc():
    import concourse.bacc as bacc
    import concourse.tile as tile
    from concourse import mybir
    from concourse.masks import make_identity

    F32 = mybir.dt.float32
    BF16 = mybir.dt.bfloat16
    AX = mybir.AxisListType
    AF = mybir.ActivationFunctionType

    nc = bacc.Bacc("TRN2", target_bir_lowering=False, debug=False, num_devices=8)

    x_d = nc.dram_tensor("x", [CIN, HW], F32, kind="ExternalInput").ap()
    xb_d = nc.dram_tensor("xb", [CIN, HW], BF16, kind="ExternalInput").ap()
    wqk_d = nc.dram_tensor("wqk", [128, 4, 128], F32, kind="ExternalInput").ap()
    wvt_d = nc.dram_tensor("wvt", [128, 4, 256], BF16, kind="ExternalInput").ap()
    wcr_d = nc.dram_tensor("wcr", [128, 2, 2, 128], F32, kind="ExternalInput").ap()
    bq_d = nc.dram_tensor("bq2", [64, 1], F32, kind="ExternalInput").ap()
    bk_d = nc.dram_tensor("bk2", [64, 1], F32, kind="ExternalInput").ap()
    bv64_d = nc.dram_tensor("bv64", [128, 2], F32, kind="ExternalInput").ap()
    bcr_d = nc.dram_tensor("bcr2", [128, 2], F32, kind="ExternalInput").ap()
    out_d = nc.dram_tensor("out", [C, QCOLS], F32, kind="ExternalOutput").ap()

    with tile.TileContext(nc) as tc:
        with (
            tc.tile_pool(name="const", bufs=1) as constp,
            tc.tile_pool(name="qkv", bufs=1) as qkvp,
            tc.tile_pool(name="pbuf", bufs=5) as pbufp,
            tc.tile_pool(name="ptbuf", bufs=3) as ptp,
            tc.tile_pool(name="attbuf", bufs=2) as attbp,
            tc.tile_pool(name="dbuf", bufs=4) as dbufp,
            tc.tile_pool(name="obuf", bufs=2) as obufp,
        ):
            # ---- constants ----
            wqk_sb = constp.tile([128, 4, 128], F32)
            wvt_sb = constp.tile([128, 4, 256], BF16)
            wcr_sb = constp.tile([128, 2, 2, 128], F32)
            bq_sb = constp.tile([64, 1], F32)
            bk_sb = constp.tile([64, 1], F32)
            bv64_sb = constp.tile([128, 2], F32)
            bcr_sb = constp.tile([128, 2], F32)
            ident = constp.tile([128, 128], BF16)
            nc.sync.dma_start(out=wqk_sb, in_=wqk_d)
            nc.sync.dma_start(out=wvt_sb, in_=wvt_d)
            nc.sync.dma_start(out=wcr_sb, in_=wcr_d)
            nc.sync.dma_start(out=bq_sb, in_=bq_d)
            nc.sync.dma_start(out=bk_sb, in_=bk_d)
            nc.sync.dma_start(out=bv64_sb, in_=bv64_d)
            nc.sync.dma_start(out=bcr_sb, in_=bcr_d)
            make_identity(nc, ident)

            q_sb = qkvp.tile([64, QCOLS], F32)       # Q for this core's queries
            k_sb = qkvp.tile([64, HW], F32)          # K, full image
            vt_sb = qkvp.tile([128, 32, 256], BF16)  # V^T, [xy-block, 128, 256]

            # ---- preamble: load x, compute Q, K, V^T ----
            with tc.tile_pool(name="xp", bufs=1) as xp, \
                 tc.tile_pool(name="ps_pre", bufs=4, space="PSUM") as ps_pre, \
                 tc.tile_pool(name="ps_prek", bufs=2, space="PSUM") as ps_prek:
                x_sb, xb_sb = [], []
                for k in range(4):
                    xk = xp.tile([128, HW], F32, tag=f"x{k}", name=f"x{k}")
                    xbk = xp.tile([128, HW], BF16, tag=f"xb{k}", name=f"xb{k}")
                    for j in range(2):
                        sl = slice(j * 2048, (j + 1) * 2048)
                        nc.sync.dma_start(out=xk[:, sl], in_=x_d[k * 128 : (k + 1) * 128, sl])
                        nc.sync.dma_start(out=xbk[:, sl], in_=xb_d[k * 128 : (k + 1) * 128, sl])
                    x_sb.append(xk)
                    xb_sb.append(xbk)

                # Q over this core's 2048 query columns (fp32)
                for n in range(4):
                    psq = ps_pre.tile([64, 512], F32, tag="pre", name="psq")
                    for k in range(4):
                        nc.tensor.matmul(
                            psq,
                            lhsT=wqk_sb[:, k, 0:64],
                            rhs=x_sb[k][:, n * 512 : (n + 1) * 512],
                            start=(k == 0),
                            stop=(k == 3),
                        )
                    nc.scalar.add(q_sb[:, n * 512 : (n + 1) * 512], psq, bq_sb)
                # K over the full image (fp32), 1024-col pairs
                for n in range(4):
                    psk = ps_prek.tile([64, 1024], F32, tag="prek", name="psk")
                    for j in range(2):
                        for k in range(4):
                            nc.tensor.matmul(
                                psk[:, j * 512 : (j + 1) * 512],
                                lhsT=wqk_sb[:, k, 64:128],
                                rhs=x_sb[k][:, (2 * n + j) * 512 : (2 * n + j + 1) * 512],
                                start=(k == 0),
                                stop=(k == 3),
                            )
                    nc.vector.tensor_scalar_add(
                        k_sb[:, n * 1024 : (n + 1) * 1024], psk, bk_sb
                    )
                # V^T (bf16 inputs, fp32 psum, bf16 out): out [xy 128, c 256]
                for i2 in range(16):
                    psv = ps_pre.tile([128, 512], F32, tag="pre", name="psv")
                    for j in range(2):
                        i = 2 * i2 + j
                        for k in range(4):
                            nc.tensor.matmul(
                                psv[:, j * 256 : (j + 1) * 256],
                                lhsT=xb_sb[k][:, i * 128 : (i + 1) * 128],
                                rhs=wvt_sb[:, k, :],
                                start=(k == 0),
                                stop=(k == 3),
                            )
                    dst = vt_sb[:, 2 * i2 : 2 * i2 + 2, :]
                    if i2 % 2 == 0:
                        nc.scalar.copy(dst, psv)
                    else:
                        nc.vector.tensor_copy(dst, psv)

            # ---- main loop over super-blocks of 512 queries ----
            with (
                tc.tile_pool(name="ps_sc", bufs=1, space="PSUM") as ps_sc,
                tc.tile_pool(name="ps_tp", bufs=2, space="PSUM") as ps_tp,
                tc.tile_pool(name="ps_att", bufs=1, space="PSUM") as ps_att,
                tc.tile_pool(name="ps_fin", bufs=2, space="PSUM") as ps_fin,
            ):
                for S in range(NSUP):
                    p_tiles = []
                    for b in range(4):
                        hw0 = (4 * S + b) * 128
                        p_b = pbufp.tile([128, HW], BF16, tag="p", name="p_b")
                        d_b = dbufp.tile([128, 64], F32, tag="d", name="d_b")
                        dr_b = dbufp.tile([128, 64], F32, tag="dr", name="dr_b")
                        drbf_b = dbufp.tile([128, 64], BF16, tag="drbf", name="drbf_b")
                        for n2 in range(4):
                            pssc = ps_sc.tile([128, 1024], F32, tag="sc", name="pssc")
                            for j in range(2):
                                nc.tensor.matmul(
                                    pssc[:, j * 512 : (j + 1) * 512],
                                    lhsT=q_sb[:, hw0 : hw0 + 128],
                                    rhs=k_sb[:, (2 * n2 + j) * 512 : (2 * n2 + j + 1) * 512],
                                    start=True,
                                    stop=True,
                                )
                            nc.scalar.activation(
                                p_b[:, n2 * 1024 : (n2 + 1) * 1024], pssc, AF.Exp
                            )
                            nc.vector.reduce_sum(
                                out=d_b[:, n2 * 16 : (n2 + 1) * 16],
                                in_=p_b[:, n2 * 1024 : (n2 + 1) * 1024].rearrange(
                                    "p (x y) -> p x y", y=64
                                ),
                                axis=AX.X,
                            )
                        nc.vector.reciprocal(dr_b, d_b)
                        nc.vector.tensor_copy(drbf_b, dr_b)
                        # attn = exp(s) / D with D broadcast over each y-group
                        dr_bc = drbf_b.unsqueeze(2).broadcast_to((128, 64, 64))
                        p3 = p_b.rearrange("p (x y) -> p x y", y=64)
                        nc.gpsimd.tensor_mul(p3, p3, dr_bc)
                        p_tiles.append(p_b)

                    # transpose attn into [xy, hw] packs: tile m = chunks
                    # {2m, 2m+1} x 4 sub-blocks, bf16
                    pt_tiles = []
                    for m in range(16):
                        pstp = ps_tp.tile([128, 1024], BF16, tag="tp", name="pstp")
                        for jj in range(2):
                            i = 2 * m + jj
                            for b in range(4):
                                c0 = jj * 512 + b * 128
                                nc.tensor.transpose(
                                    pstp[:, c0 : c0 + 128],
                                    p_tiles[b][:, i * 128 : (i + 1) * 128],
                                    ident,
                                )
                        pt = ptp.tile([128, 1024], BF16, tag="pt", name="pt")
                        if m % 2 == 0:
                            nc.scalar.copy(pt, pstp)
                        else:
                            nc.vector.tensor_copy(pt, pstp)
                        pt_tiles.append(pt)

                    # attended^T [c, hw 512]: accumulate over all 32 xy-chunks
                    att_h = [
                        ps_att.tile([128, 512], F32, tag=f"att{h}", name=f"att{h}")
                        for h in range(2)
                    ]
                    for i in range(32):
                        m, jj = divmod(i, 2)
                        rhs = pt_tiles[m][:, jj * 512 : (jj + 1) * 512]
                        for h in range(2):
                            nc.tensor.matmul(
                                att_h[h],
                                lhsT=vt_sb[:, i, h * 128 : (h + 1) * 128],
                                rhs=rhs,
                                start=(i == 0),
                                stop=(i == 31),
                            )
                    # + V-bias: sum_xy attn = 64 exactly, so bias term = 64*bv
                    attT = attbp.tile([128, 2, 512], F32, tag="attT", name="attT")
                    nc.scalar.add(attT[:, 0, :], att_h[0], bv64_sb[:, 0:1])
                    nc.vector.tensor_scalar_add(attT[:, 1, :], att_h[1], bv64_sb[:, 1:2])

                    # final projection out[co, hw] = Wcr @ att^T + bcr (fp32)
                    out_t = obufp.tile([128, 2, 512], F32, tag="out_t", name="out_t")
                    for g in range(2):
                        psf = ps_fin.tile([128, 512], F32, tag="fin", name="psf")
                        for h in range(2):
                            nc.tensor.matmul(
                                psf,
                                lhsT=wcr_sb[:, h, g, :],
                                rhs=attT[:, h, :],
                                start=(h == 0),
                                stop=(h == 1),
                            )
                        if g == 0:
                            nc.scalar.add(out_t[:, g, :], psf, bcr_sb[:, g : g + 1])
                        else:
                            nc.vector.tensor_scalar_add(
                                out_t[:, g, :], psf, bcr_sb[:, g : g + 1]
                            )
                        nc.sync.dma_start(
                            out=out_d[g * 128 : (g + 1) * 128, S * 512 : (S + 1) * 512],
                            in_=out_t[:, g, :],
                        )
    nc.compile()
    return nc


def get_nc():
    if "nc" not in _CACHE:
        _CACHE["nc"] = _build_nc()
    return _CACHE["nc"]


def make_in_maps(inputs):
    rgb = np.asarray(inputs["rgb_features"], np.float32)
    chm = np.asarray(inputs["chm_features"], np.float32)
    Wq = np.asarray(inputs["Wq"], np.float32)
    bq = np.asarray(inputs["bq"], np.float32)
    Wk = np.asarray(inputs["Wk"], np.float32)
    bk = np.asarray(inputs["bk"], np.float32)
    Wv = np.asarray(inputs["Wv"], np.float32)
    bv = np.asarray(inputs["bv"], np.float32)
    Wcr = np.asarray(inputs["Wcr"], np.float32)
    bcr = np.asarray(inputs["bcr"], np.float32)

    Wqk = np.concatenate([Wq, Wk], axis=0)  # [128, 512]
    wqk = np.ascontiguousarray(Wqk.T.reshape(4, 128, 128).transpose(1, 0, 2))
    wvt = np.ascontiguousarray(
        Wv.T.reshape(4, 128, 256).transpose(1, 0, 2)
    ).astype(ml_dtypes.bfloat16)
    wcr = np.ascontiguousarray(Wcr.T.reshape(2, 128, 2, 128).transpose(1, 0, 2, 3))
    bq2 = np.ascontiguousarray(bq.reshape(64, 1))
    bk2 = np.ascontiguousarray(bk.reshape(64, 1))
    bv64 = np.ascontiguousarray((64.0 * bv).reshape(2, 128).T)
    bcr2 = np.ascontiguousarray(bcr.reshape(2, 128).T)

    in_maps = []
    for core in range(8):
        b, par = divmod(core, 2)
        x = np.concatenate([rgb[b], chm[b]], axis=0).reshape(CIN, HW)
        if par:
            x = np.roll(x, -QCOLS, axis=1)
        x = np.ascontiguousarray(x)
        in_maps.append(
            {
                "x": x,
                "xb": x.astype(ml_dtypes.bfloat16),
                "wqk": wqk,
                "wvt": wvt,
                "wcr": wcr,
                "bq2": bq2,
                "bk2": bk2,
                "bv64": bv64,
                "bcr2": bcr2,
            }
        )
    return in_maps


def assemble(outs):
    full = np.empty((B, C, HW), np.float32)
    for core in range(8):
        b, par = divmod(core, 2)
        full[b, :, par * QCOLS : (par + 1) * QCOLS] = outs[core]
    return full.reshape(B, C, H, W)


def kernel(**inputs):
    from concourse.bass_utils import run_bass_kernel_spmd

    nc = get_nc()
    res = run_bass_kernel_spmd(nc, make_in_maps(inputs), core_ids=list(range(8)))
    return assemble([r["out"] for r in res.results])


# revision 8
# speedup vs baseline: 1.9481x; 1.3629x over previous
"""Cross-attention-concat kernel for Trainium2 (8 NeuronCores, Bass/Tile).

Math (per batch b):
  x   = concat(rgb, chm) on channels           [512, 4096]   (pixels hw = h*64+w)
  Q   = Wq x + bq ; K = Wk x + bk              [64, ...]
  V   = Wv x + bv                              [256, 4096]
  S   = Q^T K                                  [2048 hw, 4096 xy]
  A   = softmax over y within each x-group of 64 keys
  out = Wcr (A V^T)^T + bcr                    [256, 2048]

Sharding: core = (batch, H-half). The host rolls each batch's pixel axis by
2048*(core%2) so every core runs the same program with its queries at
columns 0:2048 of the rolled image (attention is invariant to the roll:
K/V/attn permute together and the roll is a multiple of the y-group 64).

Precision: scores/softmax-denominator path is fp32 (exp amplifies input
error); the attention matrix and V are bf16 (PE streams bf16 at 1 col/cycle
vs 2 for fp32, with fp32 PSUM accumulation), final projection fp32.
"""

import numpy as np
import ml_dtypes

B, C, H, W = 4, 256, 64, 64
HW = H * W               # 4096
CIN = 2 * C              # 512
QCOLS = HW // 2          # 2048 queries per core
NSUP = QCOLS // 512      # 4 super-blocks of 512 queries (4 sub-blocks of 128)

_CACHE = {}


def _build_nc():
    import concourse.bacc as bacc
    import concourse.tile as tile
    from concourse import mybir
    from concourse.masks import make_identity

    F32 = mybir.dt.float32
    BF16 = mybir.dt.bfloat16
    AX = mybir.AxisListType
    AF = mybir.ActivationFunctionType

    nc = bacc.Bacc("TRN2", target_bir_lowering=False, debug=False, num_devices=8)

    x_d = nc.dram_tensor("x", [CIN, HW], F32, kind="ExternalInput").ap()
    xb_d = nc.dram_tensor("xb", [CIN, HW], BF16, kind="ExternalInput").ap()
    wqk_d = nc.dram_tensor("wqk", [128, 4, 128], F32, kind="ExternalInput").ap()
    wvt_d = nc.dram_tensor("wvt", [128, 4, 256], BF16, kind="ExternalInput").ap()
    wcr_d = nc.dram_tensor("wcr", [128, 2, 2, 128], F32, kind="ExternalInput").ap()
    bq_d = nc.dram_tensor("bq2", [64, 1], F32, kind="ExternalInput").ap()
    bk_d = nc.dram_tensor("bk2", [64, 1], F32, kind="ExternalInput").ap()
    bv64_d = nc.dram_tensor("bv64", [128, 2], F32, kind="ExternalInput").ap()
    bcr_d = nc.dram_tensor("bcr2", [128, 2], F32, kind="ExternalInput").ap()
    out_d = nc.dram_tensor("out", [C, QCOLS], F32, kind="ExternalOutput").ap()

    with tile.TileContext(nc) as tc:
        with (
            tc.tile_pool(name="const", bufs=1) as constp,
            tc.tile_pool(name="qkv", bufs=1) as qkvp,
            tc.tile_pool(name="pbuf", bufs=5) as pbufp,
            tc.tile_pool(name="ptbuf", bufs=3) as ptp,
            tc.tile_pool(name="attbuf", bufs=2) as attbp,
            tc.tile_pool(name="dbuf", bufs=4) as dbufp,
            tc.tile_pool(name="obuf", bufs=2) as obufp,
        ):
            # ---- constants ----
            wqk_sb = constp.tile([128, 4, 128], F32)
            wvt_sb = constp.tile([128, 4, 256], BF16)
            wcr_sb = constp.tile([128, 2, 2, 128], F32)
            bq_sb = constp.tile([64, 1], F32)
            bk_sb = constp.tile([64, 1], F32)
            bv64_sb = constp.tile([128, 2], F32)
            bcr_sb = constp.tile([128, 2], F32)
            ident = constp.tile([128, 128], BF16)
            nc.sync.dma_start(out=wqk_sb, in_=wqk_d)
            nc.sync.dma_start(out=wvt_sb, in_=wvt_d)
            nc.sync.dma_start(out=wcr_sb, in_=wcr_d)
            nc.sync.dma_start(out=bq_sb, in_=bq_d)
            nc.sync.dma_start(out=bk_sb, in_=bk_d)
            nc.sync.dma_start(out=bv64_sb, in_=bv64_d)
            nc.sync.dma_start(out=bcr_sb, in_=bcr_d)
            make_identity(nc, ident)

            q_sb = qkvp.tile([64, QCOLS], F32)       # Q for this core's queries
            k_sb = qkvp.tile([64, HW], F32)          # K, full image
            vt_sb = qkvp.tile([128, 32, 256], BF16)  # V^T, [xy-block, 128, 256]

            # ---- preamble: load x, compute Q, K, V^T ----
            with tc.tile_pool(name="xp", bufs=1) as xp, \
                 tc.tile_pool(name="ps_pre", bufs=4, space="PSUM") as ps_pre, \
                 tc.tile_pool(name="ps_prek", bufs=2, space="PSUM") as ps_prek:
                x_sb, xb_sb = [], []
                for k in range(4):
                    xk = xp.tile([128, HW], F32, tag=f"x{k}", name=f"x{k}")
                    xbk = xp.tile([128, HW], BF16, tag=f"xb{k}", name=f"xb{k}")
                    for j in range(4):
                        sl = slice(j * 1024, (j + 1) * 1024)
                        nc.sync.dma_start(out=xk[:, sl], in_=x_d[k * 128 : (k + 1) * 128, sl])
                        nc.sync.dma_start(out=xbk[:, sl], in_=xb_d[k * 128 : (k + 1) * 128, sl])
                    x_sb.append(xk)
                    xb_sb.append(xbk)

                # Q over this core's 2048 query columns (fp32)
                for n in range(4):
                    psq = ps_pre.tile([64, 512], F32, tag="pre", name="psq")
                    for k in range(4):
                        nc.tensor.matmul(
                            psq,
                            lhsT=wqk_sb[:, k, 0:64],
                            rhs=x_sb[k][:, n * 512 : (n + 1) * 512],
                            start=(k == 0),
                            stop=(k == 3),
                        )
                    nc.scalar.add(q_sb[:, n * 512 : (n + 1) * 512], psq, bq_sb)
                # K over the full image (fp32), 1024-col pairs
                for n in range(4):
                    psk = ps_prek.tile([64, 1024], F32, tag="prek", name="psk")
                    for j in range(2):
                        for k in range(4):
                            nc.tensor.matmul(
                                psk[:, j * 512 : (j + 1) * 512],
                                lhsT=wqk_sb[:, k, 64:128],
                                rhs=x_sb[k][:, (2 * n + j) * 512 : (2 * n + j + 1) * 512],
                                start=(k == 0),
                                stop=(k == 3),
                            )
                    nc.vector.tensor_scalar_add(
                        k_sb[:, n * 1024 : (n + 1) * 1024], psk, bk_sb
                    )
                # V^T (bf16 inputs, fp32 psum, bf16 out): out [xy 128, c 256]
                for i2 in range(16):
                    psv = ps_pre.tile([128, 512], F32, tag="pre", name="psv")
                    for j in range(2):
                        i = 2 * i2 + j
                        for k in range(4):
                            nc.tensor.matmul(
                                psv[:, j * 256 : (j + 1) * 256],
                                lhsT=xb_sb[k][:, i * 128 : (i + 1) * 128],
                                rhs=wvt_sb[:, k, :],
                                start=(k == 0),
                                stop=(k == 3),
                            )
                    dst = vt_sb[:, 2 * i2 : 2 * i2 + 2, :]
                    if i2 % 2 == 0:
                        nc.scalar.copy(dst, psv)
                    else:
                        nc.vector.tensor_copy(dst, psv)

            # ---- main loop over super-blocks of 512 queries ----
            with (
                tc.tile_pool(name="ps_sc", bufs=2, space="PSUM") as ps_sc,
                tc.tile_pool(name="ps_tp", bufs=1, space="PSUM") as ps_tp,
                tc.tile_pool(name="ps_att", bufs=1, space="PSUM") as ps_att,
                tc.tile_pool(name="ps_fin", bufs=1, space="PSUM") as ps_fin,
            ):
                for S in range(NSUP):
                    p_tiles = []
                    for b in range(4):
                        hw0 = (4 * S + b) * 128
                        p_b = pbufp.tile([128, HW], BF16, tag="p", name="p_b")
                        d_b = dbufp.tile([128, 64], F32, tag="d", name="d_b")
                        dr_b = dbufp.tile([128, 64], F32, tag="dr", name="dr_b")
                        drbf_b = dbufp.tile([128, 64], BF16, tag="drbf", name="drbf_b")
                        for n2 in range(4):
                            pssc = ps_sc.tile([128, 1024], F32, tag="sc", name="pssc")
                            for j in range(2):
                                nc.tensor.matmul(
                                    pssc[:, j * 512 : (j + 1) * 512],
                                    lhsT=q_sb[:, hw0 : hw0 + 128],
                                    rhs=k_sb[:, (2 * n2 + j) * 512 : (2 * n2 + j + 1) * 512],
                                    start=True,
                                    stop=True,
                                )
                            nc.scalar.activation(
                                p_b[:, n2 * 1024 : (n2 + 1) * 1024], pssc, AF.Exp
                            )
                            nc.vector.reduce_sum(
                                out=d_b[:, n2 * 16 : (n2 + 1) * 16],
                                in_=p_b[:, n2 * 1024 : (n2 + 1) * 1024].rearrange(
                                    "p (x y) -> p x y", y=64
                                ),
                                axis=AX.X,
                            )
                        nc.vector.reciprocal(dr_b, d_b)
                        nc.vector.tensor_copy(drbf_b, dr_b)
                        # attn = exp(s) / D with D broadcast over each y-group
                        dr_bc = drbf_b.unsqueeze(2).broadcast_to((128, 64, 64))
                        p3 = p_b.rearrange("p (x y) -> p x y", y=64)
                        nc.vector.tensor_mul(p3[:, 0:32, :], p3[:, 0:32, :], dr_bc[:, 0:32, :])
                        nc.gpsimd.tensor_mul(p3[:, 32:64, :], p3[:, 32:64, :], dr_bc[:, 32:64, :])
                        p_tiles.append(p_b)

                    # transpose attn into [xy, hw] packs: tile m = chunks
                    # {2m, 2m+1} x 4 sub-blocks, bf16
                    pt_tiles = []
                    for m in range(16):
                        pstp = ps_tp.tile([128, 1024], BF16, tag="tp", name="pstp")
                        for jj in range(2):
                            i = 2 * m + jj
                            for b in range(4):
                                c0 = jj * 512 + b * 128
                                nc.tensor.transpose(
                                    pstp[:, c0 : c0 + 128],
                                    p_tiles[b][:, i * 128 : (i + 1) * 128],
                                    ident,
                                )
                        pt = ptp.tile([128, 1024], BF16, tag="pt", name="pt")
                        if m % 2 == 0:
                            nc.scalar.copy(pt, pstp)
                        else:
                            nc.vector.tensor_copy(pt, pstp)
                        pt_tiles.append(pt)

                    # attended^T [c, hw 512]: accumulate over all 32 xy-chunks
                    att_h = [
                        ps_att.tile([128, 512], F32, tag=f"att{h}", name=f"att{h}")
                        for h in range(2)
                    ]
                    for i in range(32):
                        m, jj = divmod(i, 2)
                        rhs = pt_tiles[m][:, jj * 512 : (jj + 1) * 512]
                        for h in range(2):
                            nc.tensor.matmul(
                                att_h[h],
                                lhsT=vt_sb[:, i, h * 128 : (h + 1) * 128],
                                rhs=rhs,
                                start=(i == 0),
                                stop=(i == 31),
                            )
                    # + V-bias: sum_xy attn = 64 exactly, so bias term = 64*bv
                    attT = attbp.tile([128, 2, 512], F32, tag="attT", name="attT")
                    nc.scalar.add(attT[:, 0, :], att_h[0], bv64_sb[:, 0:1])
                    nc.vector.tensor_scalar_add(attT[:, 1, :], att_h[1], bv64_sb[:, 1:2])

                    # final projection out[co, hw] = Wcr @ att^T + bcr (fp32)
                    out_t = obufp.tile([128, 2, 512], F32, tag="out_t", name="out_t")
                    for g in range(2):
                        psf = ps_fin.tile([128, 512], F32, tag="fin", name="psf")
                        for h in range(2):
                            nc.tensor.matmul(
                                psf,
                                lhsT=wcr_sb[:, h, g, :],
                                rhs=attT[:, h, :],
                                start=(h == 0),
                                stop=(h == 1),
                            )
                        if g == 0:
                            nc.scalar.add(out_t[:, g, :], psf, bcr_sb[:, g : g + 1])
                        else:
                            nc.vector.tensor_scalar_add(
                                out_t[:, g, :], psf, bcr_sb[:, g : g + 1]
                            )
                        nc.sync.dma_start(
                            out=out_d[g * 128 : (g + 1) * 128, S * 512 : (S + 1) * 512],
                            in_=out_t[:, g, :],
                        )
    nc.compile()
    return nc


def get_nc():
    if "nc" not in _CACHE:
        _CACHE["nc"] = _build_nc()
    return _CACHE["nc"]


def make_in_maps(inputs):
    rgb = np.asarray(inputs["rgb_features"], np.float32)
    chm = np.asarray(inputs["chm_features"], np.float32)
    Wq = np.asarray(inputs["Wq"], np.float32)
    bq = np.asarray(inputs["bq"], np.float32)
    Wk = np.asarray(inputs["Wk"], np.float32)
    bk = np.asarray(inputs["bk"], np.float32)
    Wv = np.asarray(inputs["Wv"], np.float32)
    bv = np.asarray(inputs["bv"], np.float32)
    Wcr = np.asarray(inputs["Wcr"], np.float32)
    bcr = np.asarray(inputs["bcr"], np.float32)

    Wqk = np.concatenate([Wq, Wk], axis=0)  # [128, 512]
    wqk = np.ascontiguousarray(Wqk.T.reshape(4, 128, 128).transpose(1, 0, 2))
    wvt = np.ascontiguousarray(
        Wv.T.reshape(4, 128, 256).transpose(1, 0, 2)
    ).astype(ml_dtypes.bfloat16)
    wcr = np.ascontiguousarray(Wcr.T.reshape(2, 128, 2, 128).transpose(1, 0, 2, 3))
    bq2 = np.ascontiguousarray(bq.reshape(64, 1))
    bk2 = np.ascontiguousarray(bk.reshape(64, 1))
    bv64 = np.ascontiguousarray((64.0 * bv).reshape(2, 128).T)
    bcr2 = np.ascontiguousarray(bcr.reshape(2, 128).T)

    in_maps = []
    for core in range(8):
        b, par = divmod(core, 2)
        x = np.concatenate([rgb[b], chm[b]], axis=0).reshape(CIN, HW)
        if par:
            x = np.roll(x, -QCOLS, axis=1)
        x = np.ascontiguousarray(x)
        in_maps.append(
            {
                "x": x,
                "xb": x.astype(ml_dtypes.bfloat16),
                "wqk": wqk,
                "wvt": wvt,
                "wcr": wcr,
                "bq2": bq2,
                "bk2": bk2,
                "bv64": bv64,
                "bcr2": bcr2,
            }
        )
    return in_maps


def assemble(outs):
    full = np.empty((B, C, HW), np.float32)
    for core in range(8):
        b, par = divmod(core, 2)
        full[b, :, par * QCOLS : (par + 1) * QCOLS] = outs[core]
    return full.reshape(B, C, H, W)


def kernel(**inputs):
    from concourse.bass_utils import run_bass_kernel_spmd

    nc = get_nc()
    res = run_bass_kernel_spmd(nc, make_in_maps(inputs), core_ids=list(range(8)))
    return assemble([r["out"] for r in res.results])


# revision 10
# speedup vs baseline: 1.9699x; 1.0112x over previous
"""Cross-attention-concat kernel for Trainium2 (8 NeuronCores, Bass/Tile).

Math (per batch b):
  x   = concat(rgb, chm) on channels           [512, 4096]   (pixels hw = h*64+w)
  Q   = Wq x + bq ; K = Wk x + bk              [64, ...]
  V   = Wv x + bv                              [256, 4096]
  S   = Q^T K                                  [2048 hw, 4096 xy]
  A   = softmax over y within each x-group of 64 keys
  out = Wcr (A V^T)^T + bcr                    [256, 2048]

Sharding: core = (batch, H-half). The host rolls each batch's pixel axis by
2048*(core%2) so every core runs the same program with its queries at
columns 0:2048 of the rolled image (attention is invariant to the roll:
K/V/attn permute together and the roll is a multiple of the y-group 64).

Precision: scores/softmax-denominator path is fp32 (exp amplifies input
error); the attention matrix and V are bf16 (PE streams bf16 at 1 col/cycle
vs 2 for fp32, with fp32 PSUM accumulation), final projection fp32.
"""

import numpy as np
import ml_dtypes

B, C, H, W = 4, 256, 64, 64
HW = H * W               # 4096
CIN = 2 * C              # 512
QCOLS = HW // 2          # 2048 queries per core
NSUP = QCOLS // 512      # 4 super-blocks of 512 queries (4 sub-blocks of 128)

_CACHE = {}


def _build_nc():
    import concourse.bacc as bacc
    import concourse.tile as tile
    from concourse import mybir
    from concourse.masks import make_identity

    F32 = mybir.dt.float32
    BF16 = mybir.dt.bfloat16
    AX = mybir.AxisListType
    AF = mybir.ActivationFunctionType

    nc = bacc.Bacc("TRN2", target_bir_lowering=False, debug=False, num_devices=8)

    x_d = nc.dram_tensor("x", [CIN, HW], F32, kind="ExternalInput").ap()
    wqk_d = nc.dram_tensor("wqk", [128, 4, 128], F32, kind="ExternalInput").ap()
    wvt_d = nc.dram_tensor("wvt", [128, 4, 256], F32, kind="ExternalInput").ap()
    wcr_d = nc.dram_tensor("wcr", [128, 2, 2, 128], F32, kind="ExternalInput").ap()
    bq_d = nc.dram_tensor("bq2", [64, 1], F32, kind="ExternalInput").ap()
    bk_d = nc.dram_tensor("bk2", [64, 1], F32, kind="ExternalInput").ap()
    bv64_d = nc.dram_tensor("bv64", [128, 2], F32, kind="ExternalInput").ap()
    bcr_d = nc.dram_tensor("bcr2", [128, 2], F32, kind="ExternalInput").ap()
    out_d = nc.dram_tensor("out", [C, QCOLS], F32, kind="ExternalOutput").ap()

    with tile.TileContext(nc) as tc:
        with (
            tc.tile_pool(name="const", bufs=1) as constp,
            tc.tile_pool(name="qkv", bufs=1) as qkvp,
            tc.tile_pool(name="pbuf", bufs=6) as pbufp,
            tc.tile_pool(name="ptbuf", bufs=3) as ptp,
            tc.tile_pool(name="attbuf", bufs=2) as attbp,
            tc.tile_pool(name="dbuf", bufs=4) as dbufp,
            tc.tile_pool(name="obuf", bufs=2) as obufp,
        ):
            # ---- constants ----
            wqk_sb = constp.tile([128, 4, 128], F32)
            wvt_sb = constp.tile([128, 4, 256], F32)
            wcr_sb = constp.tile([128, 2, 2, 128], F32)
            bq_sb = constp.tile([64, 1], F32)
            bk_sb = constp.tile([64, 1], F32)
            bv64_sb = constp.tile([128, 2], F32)
            bcr_sb = constp.tile([128, 2], F32)
            ident = constp.tile([128, 128], BF16)
            nc.sync.dma_start(out=wqk_sb, in_=wqk_d)
            nc.sync.dma_start(out=wvt_sb, in_=wvt_d)
            nc.sync.dma_start(out=wcr_sb, in_=wcr_d)
            nc.sync.dma_start(out=bq_sb, in_=bq_d)
            nc.sync.dma_start(out=bk_sb, in_=bk_d)
            nc.sync.dma_start(out=bv64_sb, in_=bv64_d)
            nc.sync.dma_start(out=bcr_sb, in_=bcr_d)
            make_identity(nc, ident)

            q_sb = qkvp.tile([64, QCOLS], F32)       # Q for this core's queries
            k_sb = qkvp.tile([64, HW], F32)          # K, full image
            vt_sb = qkvp.tile([128, 32, 256], BF16)  # V^T, [xy-block, 128, 256]

            # ---- preamble: load x, compute Q, K, V^T ----
            with tc.tile_pool(name="xp", bufs=1) as xp, \
                 tc.tile_pool(name="ps_pre", bufs=4, space="PSUM") as ps_pre, \
                 tc.tile_pool(name="ps_prek", bufs=2, space="PSUM") as ps_prek:
                x_sb = []
                for k in range(4):
                    xk = xp.tile([128, HW], F32, tag=f"x{k}", name=f"x{k}")
                    for j in range(4):
                        sl = slice(j * 1024, (j + 1) * 1024)
                        nc.sync.dma_start(out=xk[:, sl], in_=x_d[k * 128 : (k + 1) * 128, sl])
                    x_sb.append(xk)

                # Q over this core's 2048 query columns (fp32)
                for n in range(4):
                    psq = ps_pre.tile([64, 512], F32, tag="pre", name="psq")
                    for k in range(4):
                        nc.tensor.matmul(
                            psq,
                            lhsT=wqk_sb[:, k, 0:64],
                            rhs=x_sb[k][:, n * 512 : (n + 1) * 512],
                            start=(k == 0),
                            stop=(k == 3),
                        )
                    nc.scalar.add(q_sb[:, n * 512 : (n + 1) * 512], psq, bq_sb)
                # K over the full image (fp32), 1024-col pairs
                for n in range(4):
                    psk = ps_prek.tile([64, 1024], F32, tag="prek", name="psk")
                    for j in range(2):
                        for k in range(4):
                            nc.tensor.matmul(
                                psk[:, j * 512 : (j + 1) * 512],
                                lhsT=wqk_sb[:, k, 64:128],
                                rhs=x_sb[k][:, (2 * n + j) * 512 : (2 * n + j + 1) * 512],
                                start=(k == 0),
                                stop=(k == 3),
                            )
                    nc.vector.tensor_scalar_add(
                        k_sb[:, n * 1024 : (n + 1) * 1024], psk, bk_sb
                    )
                # V^T (bf16 inputs, fp32 psum, bf16 out): out [xy 128, c 256]
                for i2 in range(16):
                    psv = ps_pre.tile([128, 512], F32, tag="pre", name="psv")
                    for j in range(2):
                        i = 2 * i2 + j
                        for k in range(4):
                            nc.tensor.matmul(
                                psv[:, j * 256 : (j + 1) * 256],
                                lhsT=x_sb[k][:, i * 128 : (i + 1) * 128],
                                rhs=wvt_sb[:, k, :],
                                start=(k == 0),
                                stop=(k == 3),
                            )
                    dst = vt_sb[:, 2 * i2 : 2 * i2 + 2, :]
                    if i2 % 2 == 0:
                        nc.scalar.copy(dst, psv)
                    else:
                        nc.vector.tensor_copy(dst, psv)

            # ---- main loop over super-blocks of 512 queries ----
            with (
                tc.tile_pool(name="ps_sc", bufs=2, space="PSUM") as ps_sc,
                tc.tile_pool(name="ps_tp", bufs=1, space="PSUM") as ps_tp,
                tc.tile_pool(name="ps_att", bufs=1, space="PSUM") as ps_att,
                tc.tile_pool(name="ps_fin", bufs=1, space="PSUM") as ps_fin,
            ):
                def scores_softmax(S):
                    p_tiles = []
                    for b in range(4):
                        hw0 = (4 * S + b) * 128
                        p_b = pbufp.tile([128, HW], BF16, tag="p", name="p_b")
                        d_b = dbufp.tile([128, 64], F32, tag="d", name="d_b")
                        dr_b = dbufp.tile([128, 64], F32, tag="dr", name="dr_b")
                        drbf_b = dbufp.tile([128, 64], BF16, tag="drbf", name="drbf_b")
                        for n2 in range(4):
                            pssc = ps_sc.tile([128, 1024], F32, tag="sc", name="pssc")
                            for j in range(2):
                                nc.tensor.matmul(
                                    pssc[:, j * 512 : (j + 1) * 512],
                                    lhsT=q_sb[:, hw0 : hw0 + 128],
                                    rhs=k_sb[:, (2 * n2 + j) * 512 : (2 * n2 + j + 1) * 512],
                                    start=True,
                                    stop=True,
                                )
                            nc.scalar.activation(
                                p_b[:, n2 * 1024 : (n2 + 1) * 1024], pssc, AF.Exp
                            )
                            nc.vector.reduce_sum(
                                out=d_b[:, n2 * 16 : (n2 + 1) * 16],
                                in_=p_b[:, n2 * 1024 : (n2 + 1) * 1024].rearrange(
                                    "p (x y) -> p x y", y=64
                                ),
                                axis=AX.X,
                            )
                        nc.vector.reciprocal(dr_b, d_b)
                        nc.vector.tensor_copy(drbf_b, dr_b)
                        dr_bc = drbf_b.unsqueeze(2).broadcast_to((128, 64, 64))
                        p3 = p_b.rearrange("p (x y) -> p x y", y=64)
                        nc.vector.tensor_mul(p3[:, 0:32, :], p3[:, 0:32, :], dr_bc[:, 0:32, :])
                        nc.gpsimd.tensor_mul(p3[:, 32:64, :], p3[:, 32:64, :], dr_bc[:, 32:64, :])
                        p_tiles.append(p_b)
                    return p_tiles

                def attended_final(S, p_tiles):
                    pt_tiles = []
                    for m in range(16):
                        pstp = ps_tp.tile([128, 1024], BF16, tag="tp", name="pstp")
                        for jj in range(2):
                            i = 2 * m + jj
                            for b in range(4):
                                c0 = jj * 512 + b * 128
                                nc.tensor.transpose(
                                    pstp[:, c0 : c0 + 128],
                                    p_tiles[b][:, i * 128 : (i + 1) * 128],
                                    ident,
                                )
                        pt = ptp.tile([128, 1024], BF16, tag="pt", name="pt")
                        if m % 2 == 0:
                            nc.scalar.copy(pt, pstp)
                        else:
                            nc.vector.tensor_copy(pt, pstp)
                        pt_tiles.append(pt)

                    att_h = [
                        ps_att.tile([128, 512], F32, tag=f"att{h}", name=f"att{h}")
                        for h in range(2)
                    ]
                    for i in range(32):
                        m, jj = divmod(i, 2)
                        rhs = pt_tiles[m][:, jj * 512 : (jj + 1) * 512]
                        for h in range(2):
                            nc.tensor.matmul(
                                att_h[h],
                                lhsT=vt_sb[:, i, h * 128 : (h + 1) * 128],
                                rhs=rhs,
                                start=(i == 0),
                                stop=(i == 31),
                            )
                    attT = attbp.tile([128, 2, 512], F32, tag="attT", name="attT")
                    nc.scalar.add(attT[:, 0, :], att_h[0], bv64_sb[:, 0:1])
                    nc.vector.tensor_scalar_add(attT[:, 1, :], att_h[1], bv64_sb[:, 1:2])

                    out_t = obufp.tile([128, 2, 512], F32, tag="out_t", name="out_t")
                    for g in range(2):
                        psf = ps_fin.tile([128, 512], F32, tag="fin", name="psf")
                        for h in range(2):
                            nc.tensor.matmul(
                                psf,
                                lhsT=wcr_sb[:, h, g, :],
                                rhs=attT[:, h, :],
                                start=(h == 0),
                                stop=(h == 1),
                            )
                        if g == 0:
                            nc.scalar.add(out_t[:, g, :], psf, bcr_sb[:, g : g + 1])
                        else:
                            nc.vector.tensor_scalar_add(
                                out_t[:, g, :], psf, bcr_sb[:, g : g + 1]
                            )
                        nc.sync.dma_start(
                            out=out_d[g * 128 : (g + 1) * 128, S * 512 : (S + 1) * 512],
                            in_=out_t[:, g, :],
                        )

                # software pipeline: scores(S+1) is emitted before the
                # attended phase of S so the in-order PE stream never stalls
                # on the softmax trail of the current super-block.
                prev = None
                for S in range(NSUP):
                    p_tiles = scores_softmax(S)
                    if prev is not None:
                        attended_final(S - 1, prev)
                    prev = p_tiles
                attended_final(NSUP - 1, prev)
    nc.compile()
    return nc


def get_nc():
    if "nc" not in _CACHE:
        _CACHE["nc"] = _build_nc()
    return _CACHE["nc"]


def make_in_maps(inputs):
    rgb = np.asarray(inputs["rgb_features"], np.float32)
    chm = np.asarray(inputs["chm_features"], np.float32)
    Wq = np.asarray(inputs["Wq"], np.float32)
    bq = np.asarray(inputs["bq"], np.float32)
    Wk = np.asarray(inputs["Wk"], np.float32)
    bk = np.asarray(inputs["bk"], np.float32)
    Wv = np.asarray(inputs["Wv"], np.float32)
    bv = np.asarray(inputs["bv"], np.float32)
    Wcr = np.asarray(inputs["Wcr"], np.float32)
    bcr = np.asarray(inputs["bcr"], np.float32)

    Wqk = np.concatenate([Wq, Wk], axis=0)  # [128, 512]
    wqk = np.ascontiguousarray(Wqk.T.reshape(4, 128, 128).transpose(1, 0, 2))
    wvt = np.ascontiguousarray(Wv.T.reshape(4, 128, 256).transpose(1, 0, 2))
    wcr = np.ascontiguousarray(Wcr.T.reshape(2, 128, 2, 128).transpose(1, 0, 2, 3))
    bq2 = np.ascontiguousarray(bq.reshape(64, 1))
    bk2 = np.ascontiguousarray(bk.reshape(64, 1))
    bv64 = np.ascontiguousarray((64.0 * bv).reshape(2, 128).T)
    bcr2 = np.ascontiguousarray(bcr.reshape(2, 128).T)

    in_maps = []
    for core in range(8):
        b, par = divmod(core, 2)
        x = np.concatenate([rgb[b], chm[b]], axis=0).reshape(CIN, HW)
        if par:
            x = np.roll(x, -QCOLS, axis=1)
        x = np.ascontiguousarray(x)
        in_maps.append(
            {
                "x": x,
                "wqk": wqk,
                "wvt": wvt,
                "wcr": wcr,
                "bq2": bq2,
                "bk2": bk2,
                "bv64": bv64,
                "bcr2": bcr2,
            }
        )
    return in_maps


def assemble(outs):
    full = np.empty((B, C, HW), np.float32)
    for core in range(8):
        b, par = divmod(core, 2)
        full[b, :, par * QCOLS : (par + 1) * QCOLS] = outs[core]
    return full.reshape(B, C, H, W)


def kernel(**inputs):
    from concourse.bass_utils import run_bass_kernel_spmd

    nc = get_nc()
    res = run_bass_kernel_spmd(nc, make_in_maps(inputs), core_ids=list(range(8)))
    return assemble([r["out"] for r in res.results])
